# revision 12
# baseline (speedup 1.0000x reference)
"""Trainium2 Bass kernel for an LSTM decoder (nn_Decoder).

Reference computation (per step t over T=256, batch B=64, H=1024, V=1000):
    x_t    = one_hot(trg_in[:, t], V)
    gates  = [x_t, h] @ W_w.T + W_b            # [B, 4H]
    i,f,g,o = split(gates); i,f,o = sigmoid; g = tanh
    c      = f*c + i*g ; h = o*tanh(c)
    logits = h @ fc_w.T + fc_b                 # [B, V]
Output: [B, T, V].

Strategy (8 NeuronCores, tensor-parallel over the hidden dim):
  * core r owns hidden units [128r, 128(r+1)) and computes their 4 gate rows
    (512 of the 4H=4096 gate outputs) for the FULL batch each step.
  * the one-hot matmul is an embedding gather -> done on HOST into a
    per-core table gathered per (b, t): emb[t] in [B=64, 512] (bias folded in).
  * per step, each core matmuls hT(full, [1024,64] as 8 SBUF slot tiles of
    [128,64]) against its weight slice (8 K-tile matmuls, N=512, fp32),
    accumulating in PSUM on top of emb (added via an identity matmul).
  * LSTM elementwise on ScalarE/VectorE in [64p, 512f] layout; h tile is
    transposed on TensorE to [128, 64] and broadcast to all 8 cores with ONE
    remote_dma_broadcast (SWDGE) per step; slot = partition_id (logical rank),
    so no physical-core-mapping assumptions.
  * fc (logits) is tensor-parallel over V: core r computes V rows
    [125r, 125(r+1)) for step t-1 while waiting for step t's h tiles.
Double buffering everywhere; cross-step WAR hazards are covered by explicit
semaphores or by the per-step causal barrier (every core's step-t matmul
requires every core's step-(t-1) broadcast).
"""
import sys

if "/opt/trn_rl_repo" not in sys.path:
    sys.path.insert(0, "/opt/trn_rl_repo")

import numpy as np

import concourse.bass as bass
import concourse.mybir as mybir
from concourse import library_config

B = 64
H = 1024
V = 1000
T_FULL = 256
NCORES = 8
HL = H // NCORES          # 128 hidden units per core
GL = 4 * HL               # 512 gate rows per core
VL = V // NCORES          # 125 fc rows per core
F32 = mybir.dt.float32

# per-core gate column order: i, f, o, g (sigmoid block contiguous at 0:384)
# reference row blocks: i=0, f=1, g=2, o=3
GATE_BLOCKS = (0, 1, 3, 2)   # col-group -> reference gate block index

COL_I = slice(0 * HL, 1 * HL)
COL_F = slice(1 * HL, 2 * HL)
COL_O = slice(2 * HL, 3 * HL)
COL_G = slice(3 * HL, 4 * HL)


def _gate_rows(r):
    """Row indices in W_w/W_b for core r, in (i, f, o, g) col order."""
    rows = []
    for blk in GATE_BLOCKS:
        base = blk * H + HL * r
        rows.extend(range(base, base + HL))
    return np.array(rows, dtype=np.int64)


def build_bass(T=T_FULL, debug=False):
    nc = bass.Bass(target_bir_lowering=False, debug=debug)

    # ---- I/O ----
    whT_d = nc.declare_dram_parameter("whT", [128, NCORES * GL], F32, isOutput=False)
    fcT_d = nc.declare_dram_parameter("fcT", [128, NCORES * VL], F32, isOutput=False)
    fcb_d = nc.declare_dram_parameter("fcb", [1, VL], F32, isOutput=False)
    ones_d = nc.declare_dram_parameter("ones", [1, B], F32, isOutput=False)
    id64_d = nc.declare_dram_parameter("id64", [B, B], F32, isOutput=False)
    c0_d = nc.declare_dram_parameter("c0", [B, HL], F32, isOutput=False)
    hT0_d = nc.declare_dram_parameter("hT0", [128, NCORES * B], F32, isOutput=False)
    emb_d = nc.declare_dram_parameter("emb", [T, B, GL], F32, isOutput=False)
    out_d = nc.declare_dram_parameter("out", [B, T, VL], F32, isOutput=True)

    EMB_BUFS = 4

    from contextlib import ExitStack

    with ExitStack() as ctx:
        block = ctx.enter_context(nc.Block())
        s_init = ctx.enter_context(nc.semaphore("s_init"))
        s_embq = [ctx.enter_context(nc.semaphore(f"s_emb{k}")) for k in range(4)]
        s_hTq = [ctx.enter_context(nc.semaphore(f"s_hT{k}")) for k in range(2)]
        s_sndq = [ctx.enter_context(nc.semaphore(f"s_snd{k}")) for k in range(2)]
        s_prp = ctx.enter_context(nc.semaphore("s_prp"))
        s_outq = [ctx.enter_context(nc.semaphore(f"s_outd{k}")) for k in range(2)]
        s_pe_g = ctx.enter_context(nc.semaphore("s_pe_g"))
        s_pe_f = ctx.enter_context(nc.semaphore("s_pe_f"))
        s_pe_t = ctx.enter_context(nc.semaphore("s_pe_t"))
        s_sig = ctx.enter_context(nc.semaphore("s_sig"))
        s_tg = ctx.enter_context(nc.semaphore("s_tg"))
        s_tc = ctx.enter_context(nc.semaphore("s_tc"))
        s_hcp = ctx.enter_context(nc.semaphore("s_hcp"))
        s_c = ctx.enter_context(nc.semaphore("s_c"))
        s_h = ctx.enter_context(nc.semaphore("s_h"))
        s_lcp = ctx.enter_context(nc.semaphore("s_lcp"))
        # ---- SBUF ----
        whT_sb = ctx.enter_context(nc.sbuf_tensor("whT_sb", [128, NCORES * GL], F32))
        fcT_sb = ctx.enter_context(nc.sbuf_tensor("fcT_sb", [128, NCORES * VL], F32))
        fcb_sb = ctx.enter_context(nc.sbuf_tensor("fcb_sb", [1, VL], F32))
        ones_sb = ctx.enter_context(nc.sbuf_tensor("ones_sb", [1, B], F32))
        id64_sb = ctx.enter_context(nc.sbuf_tensor("id64_sb", [B, B], F32))
        emb_sb = ctx.enter_context(nc.sbuf_tensor("emb_sb", [128, EMB_BUFS * GL], F32))
        hT_a = ctx.enter_context(nc.sbuf_tensor("hT_a", [128, NCORES * B], F32))
        hT_b = ctx.enter_context(nc.sbuf_tensor("hT_b", [128, NCORES * B], F32))
        act_sb = ctx.enter_context(nc.sbuf_tensor("act_sb", [B, GL], F32))
        tc_sb = ctx.enter_context(nc.sbuf_tensor("tc_sb", [B, HL], F32))
        c_sb = ctx.enter_context(nc.sbuf_tensor("c_sb", [B, HL], F32))
        p1_sb = ctx.enter_context(nc.sbuf_tensor("p1_sb", [B, HL], F32))
        p2_sb = ctx.enter_context(nc.sbuf_tensor("p2_sb", [B, HL], F32))
        hflat_sb = ctx.enter_context(nc.sbuf_tensor("hflat_sb", [B, HL], F32))
        logit_sb = ctx.enter_context(nc.sbuf_tensor("logit_sb", [B, 2 * VL], F32))
        # ---- PSUM (each [*,512] slice = exactly one 2KB bank) ----
        pg = ctx.enter_context(nc.psum_tensor("pg", [B, 2 * 512], F32))
        pl = ctx.enter_context(nc.psum_tensor("pl", [B, 2 * 512], F32))
        pt = ctx.enter_context(nc.psum_tensor("pt", [128, 2 * 512], F32))
        hT_bufs = (hT_a, hT_b)

        # number of init DMAs (each +16) on s_init
        N_INIT = 7
        INIT_ALL = 16 * N_INIT

        # ---------------- sync engine: DMAs ----------------
        @block.sync
        def _(se: bass.BassEngine):
            se.dma_start(out=whT_sb[:, :], in_=whT_d[:, :]).then_inc(s_init, 16)
            se.dma_start(out=fcT_sb[:, :], in_=fcT_d[:, :]).then_inc(s_init, 16)
            se.dma_start(out=fcb_sb[:, :], in_=fcb_d[:, :]).then_inc(s_init, 16)
            se.dma_start(out=ones_sb[:, :], in_=ones_d[:, :]).then_inc(s_init, 16)
            se.dma_start(out=id64_sb[:, :], in_=id64_d[:, :]).then_inc(s_init, 16)
            se.dma_start(out=c_sb[:, :], in_=c0_d[:, :]).then_inc(s_init, 16)
            se.dma_start(out=hT_bufs[1][:, :], in_=hT0_d[:, :]).then_inc(s_init, 16)
            for k in range(min(EMB_BUFS, T)):
                se.dma_start(
                    out=emb_sb[:B, bass.ts(k % EMB_BUFS, GL)], in_=emb_d[k, :, :]
                ).then_inc(s_embq[k % EMB_BUFS], 16)
            for t in range(T):
                # prefetch emb(t+EMB_BUFS) once PE consumed emb(t)
                tp = t + EMB_BUFS
                if tp < T:
                    se.wait_ge(s_pe_g, t + 1)
                    se.dma_start(
                        out=emb_sb[:B, bass.ts(tp % EMB_BUFS, GL)],
                        in_=emb_d[tp, :, :],
                    ).then_inc(s_embq[tp % EMB_BUFS], 16)
                # store logits(t-1) -> needs DVE copy done (s_lcp == t)
                if t >= 1:
                    se.wait_ge(s_lcp, t)
                    se.dma_start(
                        out=out_d[:, t - 1, :],
                        in_=logit_sb[:, bass.ts((t - 1) % 2, VL)],
                    ).then_inc(s_outq[(t - 1) % 2], 16)
            # epilogue: logits(T-1)
            se.wait_ge(s_lcp, T)
            se.dma_start(
                out=out_d[:, T - 1, :], in_=logit_sb[:, bass.ts((T - 1) % 2, VL)]
            ).then_inc(s_outq[(T - 1) % 2], 16)
            se.wait_ge(s_outq[0], 16 * ((T + 1) // 2))
            se.wait_ge(s_outq[1], 16 * (T // 2))

        # ---------------- TensorE ----------------
        @block.tensor
        def _(pe: bass.BassEngine):
            pe.wait_ge(s_init, INIT_ALL)
            for t in range(T + 1):
                par = t % 2
                if t < T:
                    # gates(t): emb + sum_j hT_j @ whT_j  -> pg[par]
                    if t >= 2:
                        # pg[par] last read by sigma(t-2)/tanh_g(t-2)
                        pe.wait_ge(s_sig, t - 1)
                        pe.wait_ge(s_tg, t - 1)
                    pe.wait_ge(s_embq[t % EMB_BUFS], 16 * (t // EMB_BUFS + 1))
                    pgt = pg[:, bass.ts(par, 512)]
                    pe.matmul(
                        pgt,
                        lhsT=id64_sb[:, :],
                        rhs=emb_sb[:B, bass.ts(t % EMB_BUFS, GL)],
                        start=True,
                        stop=False,
                    )
                    if t >= 1:
                        pe.wait_ge(s_hcp, t)
                        pe.wait_ge(s_hTq[(t - 1) % 2], 14 * ((t - 1) // 2 + 1))
                    hTb = hT_bufs[(t - 1) % 2]
                    for j in range(NCORES):
                        mm = pe.matmul(
                            pgt,
                            lhsT=hTb[:, bass.ts(j, B)],
                            rhs=whT_sb[:, bass.ts(j, GL)],
                            start=False,
                            stop=(j == NCORES - 1),
                        )
                    mm.then_inc(s_pe_g, 1)
                if t >= 1:
                    # fc(t-1): logits into pl[(t-1)%2]
                    if t == T:
                        # epilogue: no gates(t) block before us -> wait h(T-1) ourselves
                        pe.wait_ge(s_hcp, t)
                        pe.wait_ge(s_hTq[(t - 1) % 2], 14 * ((t - 1) // 2 + 1))
                    if t >= 3:
                        pe.wait_ge(s_lcp, t - 2)  # pl bank free
                    plt = pl[:, bass.ts((t - 1) % 2, 512)][:, :VL]
                    hTb = hT_bufs[(t - 1) % 2]
                    pe.matmul(
                        plt, lhsT=ones_sb[:, :], rhs=fcb_sb[:, :],
                        start=True, stop=False,
                    )
                    for j in range(NCORES):
                        mm = pe.matmul(
                            plt,
                            lhsT=hTb[:, bass.ts(j, B)],
                            rhs=fcT_sb[:, bass.ts(j, VL)],
                            start=False,
                            stop=(j == NCORES - 1),
                        )
                    mm.then_inc(s_pe_f, 1)
                if t < T:
                    # transpose h(t) -> pt[par][:, :B]
                    pe.wait_ge(s_h, t + 1)
                    if t >= 2:
                        pe.wait_ge(s_hcp, t - 1)  # pt bank free
                    pe.transpose(
                        pt[:, bass.ts(par, 512)][:, :B],
                        hflat_sb[:, :],
                        id64_sb[:, :],
                    ).then_inc(s_pe_t, 1)

        # ---------------- ScalarE ----------------
        @block.scalar
        def _(ac: bass.BassEngine):
            AF = mybir.ActivationFunctionType
            ac_pid = ac.snap(ac.partition_id(), min_val=0, max_val=NCORES - 1)
            for t in range(T):
                par = t % 2
                pgt = pg[:, bass.ts(par, 512)]
                ac.wait_ge(s_pe_g, t + 1)
                if t >= 1:
                    ac.wait_ge(s_h, t)  # act_sb free (h(t-1) read sigma_o)
                ac.activation(act_sb[:, 0:3 * HL], pgt[:, 0:3 * HL], AF.Sigmoid)
                ac.drain().then_inc(s_sig, 1)
                if t >= 1:
                    ac.wait_ge(s_c, t)  # p2(t-1) read tanh_g(t-1)
                ac.activation(act_sb[:, COL_G], pgt[:, COL_G], AF.Tanh)
                ac.drain().then_inc(s_tg, 1)
                ac.wait_ge(s_c, t + 1)
                if t >= 1:
                    ac.wait_ge(s_h, t)  # tc_sb free
                ac.activation(tc_sb[:, :], c_sb[:, :], AF.Tanh)
                ac.drain().then_inc(s_tc, 1)
                # copy hT tile psum -> own slot of hT buffer [par]
                ac.wait_ge(s_pe_t, t + 1)
                if t >= 2:
                    ac.wait_ge(s_sndq[par], 16 * (t // 2))  # own slot broadcast(t-2) sent
                for k in range(NCORES):
                    with ac.If(ac_pid == k):
                        ac.copy(
                            hT_bufs[par][:, bass.ts(k, B)],
                            pt[:, bass.ts(par, 512)][:, :B],
                        )
                        ac.drain().then_inc(s_hcp, 1)

        # ---------------- VectorE ----------------
        @block.vector
        def _(ve: bass.BassEngine):
            for t in range(T + 1):
                if t < T:
                    ve.wait_ge(s_sig, t + 1)
                    ve.tensor_mul(p1_sb[:, :], act_sb[:, COL_F], c_sb[:, :])
                    ve.wait_ge(s_tg, t + 1)
                    ve.tensor_mul(p2_sb[:, :], act_sb[:, COL_G], act_sb[:, COL_I])
                    ve.drain()
                    ve.tensor_add(c_sb[:, :], p1_sb[:, :], p2_sb[:, :])
                    ve.drain().then_inc(s_c, 1)
                    ve.wait_ge(s_tc, t + 1)
                    if t >= 1:
                        ve.wait_ge(s_pe_t, t)  # hflat read by transpose(t-1)
                    ve.tensor_mul(hflat_sb[:, :], act_sb[:, COL_O], tc_sb[:, :])
                    ve.drain().then_inc(s_h, 1)
                if t >= 1:
                    # copy logits(t-1) psum -> sbuf
                    ve.wait_ge(s_pe_f, t)
                    if t >= 3:
                        ve.wait_ge(s_outq[(t - 1) % 2], 16 * ((t - 3) // 2 + 1))
                    ve.tensor_copy(
                        logit_sb[:, bass.ts((t - 1) % 2, VL)],
                        pl[:, bass.ts((t - 1) % 2, 512)][:, :VL],
                    )
                    ve.drain().then_inc(s_lcp, 1)

        # ---------------- GpSimd: broadcast ----------------
        @block.gpsimd
        def _(gp: bass.BassGpSimd):
            gp.load_library(library_config.remote_dma)
            pid = gp.snap(gp.partition_id(), min_val=0, max_val=NCORES - 1)
            rdests = [(0, k) for k in range(NCORES)]
            for t in range(T):
                par = t % 2
                # one of 8 statically-addressed preps, selected by logical rank
                for k in range(NCORES):
                    with gp.If(pid == k):
                        # rdests are XOR-deltas: delta 0 (index 0) is self -> skip
                        rd = [(0, j) if j else None for j in range(NCORES)]
                        gp.remote_dma_broadcast(
                            hT_bufs[par][:, bass.ts(k, B)],
                            hT_bufs[par][:, bass.ts(k, B)],
                            remote_sem=s_hTq[par],
                            local_sem=s_sndq[par],
                            rdests=rd,
                        ).then_inc(s_prp, 1)
                gp.wait_ge(s_prp, t + 1)
                gp.wait_ge(s_hcp, t + 1)
                gp.wait_ge(s_pe_t, t + 1)  # direct PE edge (race-detector transitivity)
                gp.trigger_dma(1)

    # extended-inst InstISA subclasses need their .instr bytes populated
    # (normally done by Bacc.compile; raw Bass must do it explicitly)
    mybir.codegen_inst_isa_subclasses(nc)
    return nc


# ----------------------------------------------------------------------------
# host side
# ----------------------------------------------------------------------------

def _prep_inputs(enc_h, enc_c, trg_in, W_w, W_b, fc_w, fc_b, T):
    """Build the 8 per-core input dicts (all float32 numpy)."""
    enc_h = np.asarray(enc_h, np.float32)
    enc_c = np.asarray(enc_c, np.float32)
    trg = np.asarray(trg_in).astype(np.int64)[:, :T]
    W_w = np.asarray(W_w, np.float32)
    W_b = np.asarray(W_b, np.float32)
    fc_w = np.asarray(fc_w, np.float32)
    fc_b = np.asarray(fc_b, np.float32)

    ones = np.ones((1, B), np.float32)
    id64 = np.eye(B, dtype=np.float32)
    # hT0[p, NCORES*b? ...] slot j at cols [B*j, B*(j+1)): hT0[p, B*j+b] = enc_h[b, 128j+p]
    hT0 = np.empty((128, NCORES * B), np.float32)
    for j in range(NCORES):
        hT0[:, B * j:B * (j + 1)] = enc_h[:, HL * j:HL * (j + 1)].T

    in_maps = []
    for r in range(NCORES):
        rows = _gate_rows(r)
        # whT: K-tile j at cols [GL*j, GL*(j+1)): whT[p, GL*j+n] = W_w[rows[n], 1000+128j+p]
        wh = W_w[rows, V:]                      # [512, 1024]
        whT = np.empty((128, NCORES * GL), np.float32)
        for j in range(NCORES):
            whT[:, GL * j:GL * (j + 1)] = wh[:, HL * j:HL * (j + 1)].T
        # emb table with bias folded: [V, 512]
        table = (W_w[rows, :V] + W_b[rows, None]).T.astype(np.float32)  # [1000, 512]
        emb = table[trg.T.reshape(-1)].reshape(T, B, GL)
        # fcT: K-tile j at cols [VL*j, VL*(j+1)): fcT[p, VL*j+m] = fc_w[125r+m, 128j+p]
        fcr = fc_w[VL * r:VL * (r + 1), :]      # [125, 1024]
        fcT = np.empty((128, NCORES * VL), np.float32)
        for j in range(NCORES):
            fcT[:, VL * j:VL * (j + 1)] = fcr[:, HL * j:HL * (j + 1)].T
        in_maps.append({
            "whT": np.ascontiguousarray(whT),
            "fcT": np.ascontiguousarray(fcT),
            "fcb": fc_b[None, VL * r:VL * (r + 1)].astype(np.float32),
            "ones": ones,
            "id64": id64,
            "c0": np.ascontiguousarray(enc_c[:, HL * r:HL * (r + 1)]),
            "hT0": hT0,
            "emb": np.ascontiguousarray(emb),
        })
    return in_maps


_NC_CACHE = {}


def kernel(enc_h, enc_c, trg_in, W_w, W_b, fc_w, fc_b):
    from concourse.bass_utils import run_bass_kernel_spmd

    T = np.asarray(trg_in).shape[1]
    in_maps = _prep_inputs(enc_h, enc_c, trg_in, W_w, W_b, fc_w, fc_b, T)
    if T not in _NC_CACHE:
        _NC_CACHE[T] = build_bass(T)
    nc = _NC_CACHE[T]
    res = run_bass_kernel_spmd(nc, in_maps, list(range(NCORES)))
    outs = [np.asarray(res.results[r]["out"]) for r in range(NCORES)]
    return np.concatenate(outs, axis=2).astype(np.float32)


if __name__ == "__main__":
    # smoke: build only
    build_bass(T=4)
    print("build ok")


# revision 16
# speedup vs baseline: 1.9924x; 1.9924x over previous
"""Trainium2 Bass kernel for an LSTM decoder (nn_Decoder).

Reference computation (per step t over T=256, batch B=64, H=1024, V=1000):
    x_t    = one_hot(trg_in[:, t], V)
    gates  = [x_t, h] @ W_w.T + W_b            # [B, 4H]
    i,f,g,o = split(gates); i,f,o = sigmoid; g = tanh
    c      = f*c + i*g ; h = o*tanh(c)
    logits = h @ fc_w.T + fc_b                 # [B, V]
Output: [B, T, V].

Strategy (8 NeuronCores, tensor-parallel over the hidden dim):
  * core r owns hidden units [128r, 128(r+1)) and computes their 4 gate rows
    (512 of the 4H=4096 gate outputs) for the FULL batch each step.
  * the one-hot matmul is an embedding gather -> done on HOST into a
    per-core table gathered per (b, t): emb[t] in [B=64, 512] (bias folded in).
  * per step, each core matmuls hT(full, [1024,64] as 8 SBUF slot tiles of
    [128,64]) against its weight slice (8 K-tile matmuls, N=512, fp32),
    accumulating in PSUM on top of emb (added via an identity matmul).
  * LSTM elementwise on ScalarE/VectorE in [64p, 512f] layout; h tile is
    transposed on TensorE to [128, 64] and broadcast to all 8 cores with ONE
    remote_dma_broadcast (SWDGE) per step; slot = partition_id (logical rank),
    so no physical-core-mapping assumptions.
  * fc (logits) is tensor-parallel over V: core r computes V rows
    [125r, 125(r+1)) for step t-1 while waiting for step t's h tiles.
Double buffering everywhere; cross-step WAR hazards are covered by explicit
semaphores or by the per-step causal barrier (every core's step-t matmul
requires every core's step-(t-1) broadcast).
"""
import sys

if "/opt/trn_rl_repo" not in sys.path:
    sys.path.insert(0, "/opt/trn_rl_repo")

import numpy as np

import concourse.bass as bass
import concourse.mybir as mybir
from concourse import library_config

B = 64
H = 1024
V = 1000
T_FULL = 256
NCORES = 8
HL = H // NCORES          # 128 hidden units per core
GL = 4 * HL               # 512 gate rows per core
VL = V // NCORES          # 125 fc rows per core
VLP = 128                 # fc tile padded to even/f32r-legal width
F32 = mybir.dt.float32
F32R = mybir.dt.float32r

# per-core gate column order: i, f, o, g (sigmoid block contiguous at 0:384)
# reference row blocks: i=0, f=1, g=2, o=3
GATE_BLOCKS = (0, 1, 3, 2)   # col-group -> reference gate block index

COL_I = slice(0 * HL, 1 * HL)
COL_F = slice(1 * HL, 2 * HL)
COL_O = slice(2 * HL, 3 * HL)
COL_G = slice(3 * HL, 4 * HL)


def _gate_rows(r):
    """Row indices in W_w/W_b for core r, in (i, f, o, g) col order."""
    rows = []
    for blk in GATE_BLOCKS:
        base = blk * H + HL * r
        rows.extend(range(base, base + HL))
    return np.array(rows, dtype=np.int64)


def build_bass(T=T_FULL, debug=False):
    nc = bass.Bass(target_bir_lowering=False, debug=debug)

    # ---- I/O ----
    whT_d = nc.declare_dram_parameter("whT", [128, NCORES * GL], F32R, isOutput=False)
    fcT_d = nc.declare_dram_parameter("fcT", [128, NCORES * VLP], F32R, isOutput=False)
    fcb_d = nc.declare_dram_parameter("fcb", [1, VLP], F32, isOutput=False)
    ones_d = nc.declare_dram_parameter("ones", [1, B], F32, isOutput=False)
    id64_d = nc.declare_dram_parameter("id64", [B, B], F32R, isOutput=False)
    id64f_d = nc.declare_dram_parameter("id64f", [B, B], F32, isOutput=False)
    c0_d = nc.declare_dram_parameter("c0", [B, HL], F32, isOutput=False)
    hT0_d = nc.declare_dram_parameter("hT0", [128, NCORES * B], F32R, isOutput=False)
    emb_d = nc.declare_dram_parameter("emb", [T, B, GL], F32R, isOutput=False)
    out_d = nc.declare_dram_parameter("out", [B, T, VL], F32, isOutput=True)

    EMB_BUFS = 4

    from contextlib import ExitStack

    with ExitStack() as ctx:
        block = ctx.enter_context(nc.Block())
        s_init = ctx.enter_context(nc.semaphore("s_init"))
        s_embq = [ctx.enter_context(nc.semaphore(f"s_emb{k}")) for k in range(4)]
        s_hTq = [ctx.enter_context(nc.semaphore(f"s_hT{k}")) for k in range(2)]
        s_sndq = [ctx.enter_context(nc.semaphore(f"s_snd{k}")) for k in range(2)]
        s_prp = ctx.enter_context(nc.semaphore("s_prp"))
        s_outq = [ctx.enter_context(nc.semaphore(f"s_outd{k}")) for k in range(2)]
        s_pe_g = ctx.enter_context(nc.semaphore("s_pe_g"))
        s_pe_f = ctx.enter_context(nc.semaphore("s_pe_f"))
        s_pe_t = ctx.enter_context(nc.semaphore("s_pe_t"))
        s_sig = ctx.enter_context(nc.semaphore("s_sig"))
        s_tg = ctx.enter_context(nc.semaphore("s_tg"))
        s_tc = ctx.enter_context(nc.semaphore("s_tc"))
        s_hcp = ctx.enter_context(nc.semaphore("s_hcp"))
        s_c = ctx.enter_context(nc.semaphore("s_c"))
        s_h = ctx.enter_context(nc.semaphore("s_h"))
        s_lcp = ctx.enter_context(nc.semaphore("s_lcp"))
        # ---- SBUF ----
        whT_sb = ctx.enter_context(nc.sbuf_tensor("whT_sb", [128, NCORES * GL], F32R))
        fcT_sb = ctx.enter_context(nc.sbuf_tensor("fcT_sb", [128, NCORES * VLP], F32R))
        fcb_sb = ctx.enter_context(nc.sbuf_tensor("fcb_sb", [1, VLP], F32))
        ones_sb = ctx.enter_context(nc.sbuf_tensor("ones_sb", [1, B], F32))
        id64_sb = ctx.enter_context(nc.sbuf_tensor("id64_sb", [B, B], F32R))
        id64f_sb = ctx.enter_context(nc.sbuf_tensor("id64f_sb", [B, B], F32))
        emb_sb = ctx.enter_context(nc.sbuf_tensor("emb_sb", [128, EMB_BUFS * GL], F32R))
        hT_a = ctx.enter_context(nc.sbuf_tensor("hT_a", [128, NCORES * B], F32R))
        hT_b = ctx.enter_context(nc.sbuf_tensor("hT_b", [128, NCORES * B], F32R))
        act_sb = ctx.enter_context(nc.sbuf_tensor("act_sb", [B, GL], F32))
        tc_sb = ctx.enter_context(nc.sbuf_tensor("tc_sb", [B, HL], F32))
        c_sb = ctx.enter_context(nc.sbuf_tensor("c_sb", [B, HL], F32))
        p1_sb = ctx.enter_context(nc.sbuf_tensor("p1_sb", [B, HL], F32))
        p2_sb = ctx.enter_context(nc.sbuf_tensor("p2_sb", [B, HL], F32))
        hflat_sb = ctx.enter_context(nc.sbuf_tensor("hflat_sb", [B, HL], F32))
        logit_sb = ctx.enter_context(nc.sbuf_tensor("logit_sb", [B, 2 * VL], F32))
        # ---- PSUM (each [*,512] slice = exactly one 2KB bank) ----
        pg = ctx.enter_context(nc.psum_tensor("pg", [B, 2 * 512], F32))
        pl = ctx.enter_context(nc.psum_tensor("pl", [B, 2 * 512], F32))
        pt = ctx.enter_context(nc.psum_tensor("pt", [128, 2 * 512], F32))
        hT_bufs = (hT_a, hT_b)

        # number of init DMAs (each +16) on s_init
        N_INIT = 8
        INIT_ALL = 16 * N_INIT

        # ---------------- sync engine: DMAs ----------------
        @block.sync
        def _(se: bass.BassEngine):
            se.dma_start(out=whT_sb[:, :], in_=whT_d[:, :]).then_inc(s_init, 16)
            se.dma_start(out=fcT_sb[:, :], in_=fcT_d[:, :]).then_inc(s_init, 16)
            se.dma_start(out=fcb_sb[:, :], in_=fcb_d[:, :]).then_inc(s_init, 16)
            se.dma_start(out=ones_sb[:, :], in_=ones_d[:, :]).then_inc(s_init, 16)
            se.dma_start(out=id64_sb[:, :], in_=id64_d[:, :]).then_inc(s_init, 16)
            se.dma_start(out=id64f_sb[:, :], in_=id64f_d[:, :]).then_inc(s_init, 16)
            se.dma_start(out=c_sb[:, :], in_=c0_d[:, :]).then_inc(s_init, 16)
            se.dma_start(out=hT_bufs[1][:, :], in_=hT0_d[:, :]).then_inc(s_init, 16)
            for k in range(min(EMB_BUFS, T)):
                se.dma_start(
                    out=emb_sb[:B, bass.ts(k % EMB_BUFS, GL)], in_=emb_d[k, :, :]
                ).then_inc(s_embq[k % EMB_BUFS], 16)
            for t in range(T):
                # prefetch emb(t+EMB_BUFS) once PE consumed emb(t)
                tp = t + EMB_BUFS
                if tp < T:
                    se.wait_ge(s_pe_g, t + 1)
                    se.dma_start(
                        out=emb_sb[:B, bass.ts(tp % EMB_BUFS, GL)],
                        in_=emb_d[tp, :, :],
                    ).then_inc(s_embq[tp % EMB_BUFS], 16)
                # store logits(t-1) -> needs DVE copy done (s_lcp == t)
                if t >= 1:
                    se.wait_ge(s_lcp, t)
                    se.dma_start(
                        out=out_d[:, t - 1, :],
                        in_=logit_sb[:, bass.ts((t - 1) % 2, VL)],
                    ).then_inc(s_outq[(t - 1) % 2], 16)
            # epilogue: logits(T-1)
            se.wait_ge(s_lcp, T)
            se.dma_start(
                out=out_d[:, T - 1, :], in_=logit_sb[:, bass.ts((T - 1) % 2, VL)]
            ).then_inc(s_outq[(T - 1) % 2], 16)
            se.wait_ge(s_outq[0], 16 * ((T + 1) // 2))
            se.wait_ge(s_outq[1], 16 * (T // 2))

        # ---------------- TensorE ----------------
        @block.tensor
        def _(pe: bass.BassEngine):
            pe.wait_ge(s_init, INIT_ALL)
            for t in range(T + 1):
                par = t % 2
                if t < T:
                    # gates(t): emb + sum_j hT_j @ whT_j  -> pg[par]
                    if t >= 2:
                        # pg[par] last read by sigma(t-2)/tanh_g(t-2)
                        pe.wait_ge(s_sig, t - 1)
                        pe.wait_ge(s_tg, t - 1)
                    pe.wait_ge(s_embq[t % EMB_BUFS], 16 * (t // EMB_BUFS + 1))
                    pgt = pg[:, bass.ts(par, 512)]
                    pe.matmul(
                        pgt,
                        lhsT=id64_sb[:, :],
                        rhs=emb_sb[:B, bass.ts(t % EMB_BUFS, GL)],
                        start=True,
                        stop=False,
                    )
                    if t >= 1:
                        pe.wait_ge(s_hcp, t)
                        pe.wait_ge(s_hTq[(t - 1) % 2], 14 * ((t - 1) // 2 + 1))
                    hTb = hT_bufs[(t - 1) % 2]
                    for j in range(NCORES):
                        mm = pe.matmul(
                            pgt,
                            lhsT=hTb[:, bass.ts(j, B)],
                            rhs=whT_sb[:, bass.ts(j, GL)],
                            start=False,
                            stop=(j == NCORES - 1),
                        )
                    mm.then_inc(s_pe_g, 1)
                if t >= 1:
                    # fc(t-1): logits into pl[(t-1)%2]
                    if t == T:
                        # epilogue: no gates(t) block before us -> wait h(T-1) ourselves
                        pe.wait_ge(s_hcp, t)
                        pe.wait_ge(s_hTq[(t - 1) % 2], 14 * ((t - 1) // 2 + 1))
                    if t >= 3:
                        pe.wait_ge(s_lcp, t - 2)  # pl bank free
                    plt = pl[:, bass.ts((t - 1) % 2, 512)][:, :VLP]
                    hTb = hT_bufs[(t - 1) % 2]
                    pe.matmul(
                        plt, lhsT=ones_sb[:, :],
                        rhs=fcb_sb[:, :],
                        start=True, stop=False,
                    )
                    for j in range(NCORES):
                        mm = pe.matmul(
                            plt,
                            lhsT=hTb[:, bass.ts(j, B)],
                            rhs=fcT_sb[:, bass.ts(j, VLP)],
                            start=False,
                            stop=(j == NCORES - 1),
                        )
                    mm.then_inc(s_pe_f, 1)
                if t < T:
                    # transpose h(t) -> pt[par][:, :B]
                    pe.wait_ge(s_h, t + 1)
                    if t >= 2:
                        pe.wait_ge(s_hcp, t - 1)  # pt bank free
                    pe.transpose(
                        pt[:, bass.ts(par, 512)][:, :B],
                        hflat_sb[:, :],
                        id64f_sb[:, :],
                    ).then_inc(s_pe_t, 1)

        # ---------------- ScalarE ----------------
        @block.scalar
        def _(ac: bass.BassEngine):
            AF = mybir.ActivationFunctionType
            ac_pid = ac.snap(ac.partition_id(), min_val=0, max_val=NCORES - 1)
            for t in range(T):
                par = t % 2
                pgt = pg[:, bass.ts(par, 512)]
                ac.wait_ge(s_pe_g, t + 1)
                if t >= 1:
                    ac.wait_ge(s_h, t)  # act_sb free (h(t-1) read sigma_o)
                ac.activation(act_sb[:, 0:3 * HL], pgt[:, 0:3 * HL], AF.Sigmoid)
                ac.drain().then_inc(s_sig, 1)
                if t >= 1:
                    ac.wait_ge(s_c, t)  # p2(t-1) read tanh_g(t-1)
                ac.activation(act_sb[:, COL_G], pgt[:, COL_G], AF.Tanh)
                ac.drain().then_inc(s_tg, 1)
                ac.wait_ge(s_c, t + 1)
                if t >= 1:
                    ac.wait_ge(s_h, t)  # tc_sb free
                ac.activation(tc_sb[:, :], c_sb[:, :], AF.Tanh)
                ac.drain().then_inc(s_tc, 1)
                # copy hT tile psum -> own slot of hT buffer [par]
                ac.wait_ge(s_pe_t, t + 1)
                if t >= 2:
                    ac.wait_ge(s_sndq[par], 16 * (t // 2))  # own slot broadcast(t-2) sent
                for k in range(NCORES):
                    with ac.If(ac_pid == k):
                        ac.copy(
                            hT_bufs[par][:, bass.ts(k, B)],
                            pt[:, bass.ts(par, 512)][:, :B],
                        )
                        ac.drain().then_inc(s_hcp, 1)

        # ---------------- VectorE ----------------
        @block.vector
        def _(ve: bass.BassEngine):
            for t in range(T + 1):
                if t < T:
                    ve.wait_ge(s_sig, t + 1)
                    ve.tensor_mul(p1_sb[:, :], act_sb[:, COL_F], c_sb[:, :])
                    ve.wait_ge(s_tg, t + 1)
                    ve.tensor_mul(p2_sb[:, :], act_sb[:, COL_G], act_sb[:, COL_I])
                    ve.drain()
                    ve.tensor_add(c_sb[:, :], p1_sb[:, :], p2_sb[:, :])
                    ve.drain().then_inc(s_c, 1)
                    ve.wait_ge(s_tc, t + 1)
                    if t >= 1:
                        ve.wait_ge(s_pe_t, t)  # hflat read by transpose(t-1)
                    ve.tensor_mul(hflat_sb[:, :], act_sb[:, COL_O], tc_sb[:, :])
                    ve.drain().then_inc(s_h, 1)
                if t >= 1:
                    # copy logits(t-1) psum -> sbuf
                    ve.wait_ge(s_pe_f, t)
                    if t >= 3:
                        ve.wait_ge(s_outq[(t - 1) % 2], 16 * ((t - 3) // 2 + 1))
                    ve.tensor_copy(
                        logit_sb[:, bass.ts((t - 1) % 2, VL)],
                        pl[:, bass.ts((t - 1) % 2, 512)][:, :VL],
                    )
                    ve.drain().then_inc(s_lcp, 1)

        # ---------------- GpSimd: broadcast ----------------
        @block.gpsimd
        def _(gp: bass.BassGpSimd):
            gp.load_library(library_config.remote_dma)
            pid = gp.snap(gp.partition_id(), min_val=0, max_val=NCORES - 1)
            rdests = [(0, k) for k in range(NCORES)]
            for t in range(T):
                par = t % 2
                # one of 8 statically-addressed preps, selected by logical rank
                for k in range(NCORES):
                    with gp.If(pid == k):
                        # rdests are XOR-deltas: delta 0 (index 0) is self -> skip
                        rd = [(0, j) if j else None for j in range(NCORES)]
                        gp.remote_dma_broadcast(
                            hT_bufs[par][:, bass.ts(k, B)],
                            hT_bufs[par][:, bass.ts(k, B)],
                            remote_sem=s_hTq[par],
                            local_sem=s_sndq[par],
                            rdests=rd,
                        ).then_inc(s_prp, 1)
                gp.wait_ge(s_prp, t + 1)
                gp.wait_ge(s_hcp, t + 1)
                gp.wait_ge(s_pe_t, t + 1)  # direct PE edge (race-detector transitivity)
                gp.trigger_dma(1)

    # extended-inst InstISA subclasses need their .instr bytes populated
    # (normally done by Bacc.compile; raw Bass must do it explicitly)
    mybir.codegen_inst_isa_subclasses(nc)
    return nc


# ----------------------------------------------------------------------------
# host side
# ----------------------------------------------------------------------------

def _prep_inputs(enc_h, enc_c, trg_in, W_w, W_b, fc_w, fc_b, T):
    """Build the 8 per-core input dicts (all float32 numpy)."""
    enc_h = np.asarray(enc_h, np.float32)
    enc_c = np.asarray(enc_c, np.float32)
    trg = np.asarray(trg_in).astype(np.int64)[:, :T]
    W_w = np.asarray(W_w, np.float32)
    W_b = np.asarray(W_b, np.float32)
    fc_w = np.asarray(fc_w, np.float32)
    fc_b = np.asarray(fc_b, np.float32)

    ones = np.ones((1, B), np.float32)
    id64 = np.eye(B, dtype=np.float32)
    # hT0[p, NCORES*b? ...] slot j at cols [B*j, B*(j+1)): hT0[p, B*j+b] = enc_h[b, 128j+p]
    hT0 = np.empty((128, NCORES * B), np.float32)
    for j in range(NCORES):
        hT0[:, B * j:B * (j + 1)] = enc_h[:, HL * j:HL * (j + 1)].T

    in_maps = []
    for r in range(NCORES):
        rows = _gate_rows(r)
        # whT: K-tile j at cols [GL*j, GL*(j+1)): whT[p, GL*j+n] = W_w[rows[n], 1000+128j+p]
        wh = W_w[rows, V:]                      # [512, 1024]
        whT = np.empty((128, NCORES * GL), np.float32)
        for j in range(NCORES):
            whT[:, GL * j:GL * (j + 1)] = wh[:, HL * j:HL * (j + 1)].T
        # emb table with bias folded: [V, 512]
        table = (W_w[rows, :V] + W_b[rows, None]).T.astype(np.float32)  # [1000, 512]
        emb = table[trg.T.reshape(-1)].reshape(T, B, GL)
        # fcT: K-tile j at cols [VL*j, VL*(j+1)): fcT[p, VL*j+m] = fc_w[125r+m, 128j+p]
        fcr = np.zeros((VLP, H), np.float32)
        fcr[:VL] = fc_w[VL * r:VL * (r + 1), :]
        fcT = np.zeros((128, NCORES * VLP), np.float32)
        for j in range(NCORES):
            fcT[:, VLP * j:VLP * (j + 1)] = fcr[:, HL * j:HL * (j + 1)].T
        in_maps.append({
            "whT": np.ascontiguousarray(whT),
            "fcT": np.ascontiguousarray(fcT),
            "fcb": np.pad(fc_b[VL * r:VL * (r + 1)].astype(np.float32), (0, VLP - VL))[None, :],
            "ones": ones,
            "id64": id64,
            "id64f": id64,
            "c0": np.ascontiguousarray(enc_c[:, HL * r:HL * (r + 1)]),
            "hT0": hT0,
            "emb": np.ascontiguousarray(emb),
        })
    return in_maps


_NC_CACHE = {}


def kernel(enc_h, enc_c, trg_in, W_w, W_b, fc_w, fc_b):
    from concourse.bass_utils import run_bass_kernel_spmd

    T = np.asarray(trg_in).shape[1]
    in_maps = _prep_inputs(enc_h, enc_c, trg_in, W_w, W_b, fc_w, fc_b, T)
    if T not in _NC_CACHE:
        _NC_CACHE[T] = build_bass(T)
    nc = _NC_CACHE[T]
    res = run_bass_kernel_spmd(nc, in_maps, list(range(NCORES)))
    outs = [np.asarray(res.results[r]["out"]) for r in range(NCORES)]
    return np.concatenate(outs, axis=2).astype(np.float32)


if __name__ == "__main__":
    # smoke: build only
    build_bass(T=4)
    print("build ok")
